# revision 12
# baseline (speedup 1.0000x reference)
"""Trainium2 Bass kernel for the AGSOM sequential scan problem.

Problem: embeddings [512, 1024, 64], nodes [4,4,64]. For each batch lane, run
3 sequential passes over the 1024 items; each step finds the BMU (argmin
euclidean distance over the 16 grid nodes), then pulls the BMU's 4-connected
neighbors toward the item by LR=0.01. Output = per-lane sum of final grid
nodes -> [512, 64].

Sharding: pure data parallel, 64 batch lanes per NeuronCore across 8 cores.

Per-core layout ("layout B"): SBUF partition p = h*64 + b, where b is the
lane and h selects half of the grid (h=0: rows 0-1 = nodes 0..7, h=1: rows
2-3 = nodes 8..15). Grid state G is [128, 8*64] fp32. Each step:
  DVE:  diff = e - g; sq = diff^2; d2 = segmented reduce -> [128, 8]
  PE :  two selection matmuls gather both halves' d2 into PSUM [128, 16]
        (the only cross-partition exchange; lanes never mix)
  DVE:  m = min(d2full); oh = (d2full == m)*LR written into a zero-padded
        6x6 tile; 4-neighbor stencil via two shifted adds; per-half local
        mask; update g += mask * diff.
Items are DMA'd in double-buffered windows; the schedule runs under hardware
Fori loops with semaphore pipelining between DVE / PE / DMA. Same-engine
ordering relies on per-engine in-order execution (DVE drains between ops);
build with unroll=True, paranoid=True for a sim-checkable fully-synced graph.
"""
import numpy as np

LR = 0.01

# Fixed full-problem config
B, T, D = 512, 1024, 64
NCORES = 8
LANES = B // NCORES  # 64 lanes per core
NPASSES = 3
WIN = 128  # items per DMA window


def build_nc(n_items=T, n_passes=NPASSES, w=WIN, unroll=False, sync="mixd48",
             debug_steps=None, no_pe=False, pe_warm=0, pool=False,
             pool_nodes=2, one_mm=False, red_split=False):
    """sync: 'drain' (HW-correct engine-pipeline barriers), 'sem' (full
    semaphore chain; sim-race-detector-checkable; requires unroll), or
    'none' (no same-engine barriers; incorrect on HW, for perf probing)."""
    from contextlib import ExitStack
    from concourse import bass, mybir

    paranoid = sync == "sem"
    assert not (paranoid and not unroll), "sync='sem' requires unroll"
    assert debug_steps is None or unroll, "debug_steps requires unroll"

    f32 = mybir.dt.float32
    ALU = mybir.AluOpType
    AX = mybir.AxisListType
    DVE = mybir.EngineType.DVE
    PE = mybir.EngineType.PE
    SP = mybir.EngineType.SP
    POOLE = mybir.EngineType.Pool
    assert not (pool and unroll), "pool slicing implemented for Fori mode only"
    assert not (pool and no_pe)
    assert 1 <= pool_nodes <= 7
    assert not (red_split and not pool), "red_split only meaningful with pool"

    wpp = n_items // w            # windows per pass
    assert wpp * w == n_items
    nw = n_passes * wpp           # total windows
    assert nw % 2 == 0, "need an even number of windows for the pair loop"
    npairs = nw // 2
    nsteps = n_passes * n_items
    EW = 2 * w * 64               # E row length (2 slots)

    nc = bass.Bass(target_bir_lowering=False)

    embd = nc.declare_dram_parameter("emb", [LANES, n_items * 64], f32, isOutput=False)
    g0d = nc.declare_dram_parameter("g0", [128, 512], f32, isOutput=False)
    s0d = nc.declare_dram_parameter("sel0", [128, 128], f32, isOutput=False)
    s1d = nc.declare_dram_parameter("sel1", [128, 128], f32, isOutput=False)
    outd = nc.declare_dram_parameter("out", [128, 64], f32, isOutput=True)
    dbg_outs = {}
    if debug_steps is not None:
        for nm, cols in [("dG", 512), ("ddiff", 512), ("dsq", 512),
                         ("dd2", 16 if one_mm else 8),
                         ("dd2f", 16), ("dmt", 1), ("dohp", 36), ("dt1", 16),
                         ("dt2", 16), ("dmkL", 8)]:
            dbg_outs[nm] = nc.declare_dram_parameter(nm, [128, cols], f32,
                                                     isOutput=True)

    with ExitStack() as ctx:
        sem_d2 = ctx.enter_context(nc.semaphore("sem_d2"))
        sem_sel = ctx.enter_context(nc.semaphore("sem_sel"))
        sem_done = ctx.enter_context(nc.semaphore("sem_done"))
        dma_sem = ctx.enter_context(nc.semaphore("dma_sem"))
        sem_psq = ctx.enter_context(nc.semaphore("sem_psq")) if pool else None
        sem_mk = ctx.enter_context(nc.semaphore("sem_mk")) if pool else None
        sem_pg = ctx.enter_context(nc.semaphore("sem_pg")) if pool else None
        vch = ctx.enter_context(nc.semaphore("vch")) if paranoid else None
        esems = [ctx.enter_context(nc.semaphore(f"es{i}")) for i in range(6)] \
            if sync == "selfsem" else None

        E = ctx.enter_context(nc.sbuf_tensor("E", [128, EW], f32))
        G = ctx.enter_context(nc.sbuf_tensor("G", [128, 512], f32))
        diff = ctx.enter_context(nc.sbuf_tensor("diff", [128, 512], f32))
        sq = ctx.enter_context(nc.sbuf_tensor("sq", [128, 512], f32))
        d2 = ctx.enter_context(nc.sbuf_tensor("d2", [128, 16 if one_mm else 8],
                                              f32))
        d2p = ctx.enter_context(nc.sbuf_tensor("d2p", [128, 16], f32))
        mt = ctx.enter_context(nc.sbuf_tensor("mt", [128, 1], f32))
        ohp = ctx.enter_context(nc.sbuf_tensor("ohp", [128, 36], f32))
        t1 = ctx.enter_context(nc.sbuf_tensor("t1", [128, 16], f32))
        t2 = ctx.enter_context(nc.sbuf_tensor("t2", [128, 16], f32))
        mkL = ctx.enter_context(nc.sbuf_tensor("mkL", [128, 8], f32))
        s17 = ctx.enter_context(nc.sbuf_tensor("s17", [128, 17], f32))
        eq17 = ctx.enter_context(nc.sbuf_tensor("eq17", [128, 17], f32))
        zz16 = ctx.enter_context(nc.sbuf_tensor("zz16", [128, 16], f32))
        sel0 = ctx.enter_context(nc.sbuf_tensor("sel0sb", [128, 128], f32))
        sel1 = ctx.enter_context(nc.sbuf_tensor("sel1sb", [128, 128], f32))
        osb = ctx.enter_context(nc.sbuf_tensor("osb", [128, 64], f32))

        d2f = ctx.enter_context(nc.psum_tensor("d2f", [128, 16], f32))
        pwrm = ctx.enter_context(nc.psum_tensor("pwrm", [128, 8], f32))
        d2fsb = ctx.enter_context(nc.sbuf_tensor("d2fsb", [128, 16], f32))

        v = nc.vector
        pe = nc.tensor
        sp = nc.sync

        # ---- static APs ----
        g_3d = bass.AP(G, 0, [[512, 128], [64, 8], [1, 64]])
        g_2d = G[:, :]
        diff_3d = bass.AP(diff, 0, [[512, 128], [64, 8], [1, 64]])
        diff_2d = diff[:, :]
        sq_2d = sq[:, :]
        sq_3d = bass.AP(sq, 0, [[512, 128], [64, 8], [1, 64]])
        sq_4d = bass.AP(sq, 0, [[512, 128], [64, 8], [32, 2], [1, 32]])
        d2p_out = bass.AP(d2p, 0, [[16, 128], [2, 8], [1, 2]])
        d2p_a = bass.AP(d2p, 0, [[16, 128], [2, 8]])
        d2p_b = bass.AP(d2p, 1, [[16, 128], [2, 8]])
        upd_3d = sq_3d  # upd reuses the sq tile
        upd_2d = sq_2d
        mkL_b = bass.AP(mkL, 0, [[8, 128], [1, 8], [0, 64]])
        ohp_int = bass.AP(ohp, 7, [[36, 128], [6, 4], [1, 4]])
        d2f_44 = bass.AP(d2f, 0, [[16, 128], [4, 4], [1, 4]])
        d2f_2d = d2f[:, :]
        # stencil source views on the padded 6x6 tile
        s_y0 = bass.AP(ohp, 6, [[36, 128], [6, 4], [1, 4]])    # oh[x, y-1]
        s_y2 = bass.AP(ohp, 8, [[36, 128], [6, 4], [1, 4]])    # oh[x, y+1]
        s_x0 = bass.AP(ohp, 1, [[36, 128], [6, 4], [1, 4]])    # oh[x-1, y]
        s_x2 = bass.AP(ohp, 13, [[36, 128], [6, 4], [1, 4]])   # oh[x+1, y]
        t1_44 = bass.AP(t1, 0, [[16, 128], [4, 4], [1, 4]])
        t2_44 = bass.AP(t2, 0, [[16, 128], [4, 4], [1, 4]])
        t1_h0 = t1[0:64, 0:8]
        t2_h0 = t2[0:64, 0:8]
        mk_h0 = mkL[0:64, 0:8]
        t1_h1 = t1[64:128, 8:16]
        t2_h1 = t2[64:128, 8:16]
        mk_h1 = mkL[64:128, 0:8]
        g_dn = bass.AP(G, 0, [[512, 128], [1, 64], [64, 8]])
        # --- pool slicing: DVE owns nodes 0..nd-1, POOL owns nodes nd..7 ---
        nd = 8 - pool_nodes
        npo = pool_nodes
        NB = nd * 64
        g_3dD = bass.AP(G, 0, [[512, 128], [64, nd], [1, 64]])
        diff_3dD = bass.AP(diff, 0, [[512, 128], [64, nd], [1, 64]])
        upd_3dD = bass.AP(sq, 0, [[512, 128], [64, nd], [1, 64]])
        mkL_bD = bass.AP(mkL, 0, [[8, 128], [1, nd], [0, 64]])
        g_3dP = bass.AP(G, NB, [[512, 128], [64, npo], [1, 64]])
        diff_3dP = bass.AP(diff, NB, [[512, 128], [64, npo], [1, 64]])
        upd_3dP = bass.AP(sq, NB, [[512, 128], [64, npo], [1, 64]])
        mkL_bP = bass.AP(mkL, nd, [[8, 128], [1, npo], [0, 64]])
        # split-reduce APs: DVE-node segment and POOL-node segment
        d2p_outD = bass.AP(d2p, 0, [[16, 128], [2, nd], [1, 2]])
        sq_4dD = bass.AP(sq, 0, [[512, 128], [64, nd], [32, 2], [1, 32]])
        d2p_outP = bass.AP(d2p, 2 * nd, [[16, 128], [2, npo], [1, 2]])
        sq_4dP = bass.AP(sq, NB, [[512, 128], [64, npo], [32, 2], [1, 32]])
        # one_mm: split pair-adds into the zero-padded d2w tile (reuses d2)
        d2p_aH0 = bass.AP(d2p, 0, [[16, 64], [2, 8]])
        d2p_bH0 = bass.AP(d2p, 1, [[16, 64], [2, 8]])
        d2p_aH1 = bass.AP(d2p, 64 * 16, [[16, 64], [2, 8]])
        d2p_bH1 = bass.AP(d2p, 64 * 16 + 1, [[16, 64], [2, 8]])
        s17_out = s17[:, 1:17]
        eq17_a = bass.AP(eq17, 1, [[17, 128], [4, 4], [1, 4]])
        eq17_b = bass.AP(eq17, 0, [[17, 128], [4, 4], [1, 4]])
        INF0 = 3.0e38

        # Same-engine ordering between dependent DVE ops. On TRN2 hardware,
        # back-to-back DVE instructions can overlap in the pipe: a reader can
        # overtake the previous writer, so every RAW/WAR edge needs a barrier.
        # 'sem': then_inc/wait_ge chain (visible to the sim race detector).
        # 'drain': engine pipeline flush instruction (cheap, HW-correct).
        vcount = [0]

        def VC(instr, inc=True):
            if paranoid and inc:
                instr.then_inc(vch, 1)
                vcount[0] += 1
            return instr

        def VW(hazard=True):
            """hazard=True: real short-distance RAW edge (drain on HW).
            hazard=False: safe-by-streaming-lag on HW, but still a
            dependency edge the sim's race detector must see."""
            if paranoid and vcount[0] > 0:
                v.wait_ge(vch, vcount[0])
            elif sync == "drain" and hazard:
                v.drain()
            elif sync == "nop" and hazard:
                v.nop(cycle_cnt=160, nofuse=True)


        def emit_step(e_off, step_idx):
            """One SOM step. Barriers only on hazardous same-engine edges
            (short-distance RAW): either a drain, or in selfsem mode a
            then_inc on the producer + wait_ge(edge_sem, step+1) before the
            consumer. Long streaming ops reading the previous op's output in
            the same element order are safe with no barrier at all."""
            if not (unroll or no_pe):
                gsv = step_idx
                v.reg_add(gsv, gsv, 1)

            def HZ(instr, k):
                # producer side of hazard edge k
                if sync == "selfsem":
                    instr.then_inc(esems[k], 1)
                return instr

            def HW_(k):
                # consumer side of hazard edge k
                if paranoid and vcount[0] > 0:
                    v.wait_ge(vch, vcount[0])
                elif sync == "selfsem":
                    if unroll:
                        v.wait_ge(esems[k], step_idx + 1)
                    else:
                        v.wait_ge(esems[k], gsv)
                elif sync == "drain":
                    v.drain()
                elif sync.startswith("nopd"):
                    v.nop(cycle_cnt=int(sync[4:]), nofuse=True)
                elif sync.startswith("mixd"):
                    # full flush only after the 512-wide reduce (edge 0);
                    # short nop covers the small-op RAW edges
                    if k == 0:
                        v.drain()
                    else:
                        v.nop(cycle_cnt=int(sync[4:]), nofuse=True)
                elif sync.startswith("mix2_"):
                    # mix2_<A>_<B>: nop(A) after the reduce, nop(B) elsewhere
                    a, b = sync[5:].split("_")
                    v.nop(cycle_cnt=int(a) if k == 0 else int(b), nofuse=True)

            e_b = bass.AP(E, e_off, [[EW, 128], [0, 8], [1, 64]])
            if pool:
                e_bD = bass.AP(E, e_off, [[EW, 128], [0, nd], [1, 64]])
                VW(False); VC(v.tensor_tensor(diff_3dD, e_bD, g_3dD,
                                              op=ALU.subtract))
                VW(False); VC(v.tensor_tensor(sq[:, 0:NB], diff[:, 0:NB],
                                              diff[:, 0:NB], op=ALU.mult))
                if red_split:
                    # DVE-node reduce needs only DVE's sq; overlaps POOL tail
                    VW(False)
                    VC(v.tensor_reduce(d2p_outD, sq_4dD, axis=AX.X, op=ALU.add))
                    v.wait_ge(sem_psq, gsv)
                    VW(False)
                    HZ(VC(v.tensor_reduce(d2p_outP, sq_4dP, axis=AX.X,
                                          op=ALU.add)), 0)
                    HW_(0)
                else:
                    # full reduce reads POOL's sq slice too
                    v.wait_ge(sem_psq, gsv)
                    VW(False)
                    HZ(VC(v.tensor_reduce(d2p_out, sq_4d, axis=AX.X,
                                          op=ALU.add)), 0)
                    HW_(0)
            else:
                VW(False); VC(v.tensor_tensor(diff_3d, e_b, g_3d, op=ALU.subtract))
                VW(False); VC(v.tensor_tensor(sq_2d, diff_2d, diff_2d, op=ALU.mult))
                VW(False)
                HZ(VC(v.tensor_reduce(d2p_out, sq_4d, axis=AX.X, op=ALU.add)), 0)
                HW_(0)
            # sem_d2 inc carries this step's progress to PE and the DMA engine
            if one_mm:
                # split pair-adds land each half's 8 sums in its own column
                # range of the zero-padded d2w tile; one K=128 matmul then
                # gathers both halves (complement columns stay 0, so the
                # PSUM sum adds an exact +0.0)
                VC(v.tensor_tensor(d2[0:64, 0:8], d2p_aH0, d2p_bH0,
                                   op=ALU.add))
                VW(False)
                v.tensor_tensor(d2[64:128, 8:16], d2p_aH1, d2p_bH1,
                                op=ALU.add).then_inc(sem_d2, 1)
            else:
                v.tensor_tensor(d2[:, :], d2p_a, d2p_b,
                                op=ALU.add).then_inc(sem_d2, 1)
            if no_pe:
                pass
            elif unroll:
                v.wait_ge(sem_sel, step_idx + 1)
            else:
                v.wait_ge(sem_sel, gsv)
            # running-min scan over the 16 gathered distances; s17[:,16] is
            # the global min, and the first position where the running min
            # equals it is the argmin with first-index tie-breaking.
            VW(False)
            HZ(VC(v.tensor_tensor_scan(s17_out, d2f_2d, zz16[:, :], INF0,
                                       op0=ALU.min, op1=ALU.bypass)), 1)
            HW_(1)
            HZ(VC(v.tensor_scalar(eq17[:, :], s17[:, :], s17[:, 16:17], LR,
                                  op0=ALU.is_equal, op1=ALU.mult)), 2)
            HW_(2)
            HZ(VC(v.tensor_tensor(ohp_int, eq17_a, eq17_b, op=ALU.subtract)), 3)
            HW_(3)
            VC(v.tensor_tensor(t1_44, s_y0, s_y2, op=ALU.add))
            VW(False)
            HZ(VC(v.tensor_tensor(t2_44, s_x0, s_x2, op=ALU.add)), 4)
            HW_(4)
            VC(v.tensor_tensor(mk_h0, t1_h0, t2_h0, op=ALU.add))
            VW(False)
            mk1i = HZ(VC(v.tensor_tensor(mk_h1, t1_h1, t2_h1, op=ALU.add)), 5)
            if pool:
                mk1i.then_inc(sem_mk, 1)
            HW_(5)
            if pool:
                VC(v.tensor_tensor(upd_3dD, diff_3dD, mkL_bD, op=ALU.mult))
                VW(False)
                VC(v.tensor_tensor(G[:, 0:NB], G[:, 0:NB], sq[:, 0:NB],
                                   op=ALU.add))
            else:
                VC(v.tensor_tensor(upd_3d, diff_3d, mkL_b, op=ALU.mult))
                # updadd reads upd in the same element order it was written,
                # with a full-op lag - streaming-safe without a drain
                VW(False)
                VC(v.tensor_tensor(g_2d, g_2d, upd_2d, op=ALU.add))

        def emit_window_load(k, wreg_or_w, pair=None):
            """SP-side load of window w into slot k (static k)."""
            dst0 = bass.AP(E, k * (w * 64), [[EW, 64], [1, w * 64]])
            dst1 = bass.AP(E, 64 * EW + k * (w * 64), [[EW, 64], [1, w * 64]])
            if unroll:
                wi = wreg_or_w
                sp.wait_ge(sem_d2, max(0, (wi - 1) * w))
                soff_i = (wi % wpp) * (w * 64)
                src0 = bass.AP(embd, soff_i, [[n_items * 64, 64], [1, w * 64]])
                src1 = bass.AP(embd, soff_i, [[n_items * 64, 64], [1, w * 64]])
                base = 48 + 32 * wi
                sp.dma_start(out=dst0, in_=src0).then_inc(dma_sem, 16)
                sp.wait_ge(dma_sem, base + 16)
                sp.dma_start(out=dst1, in_=src1).then_inc(dma_sem, 16)
                sp.wait_ge(dma_sem, base + 32)
            else:
                wreg, widx, soff, thr, p = wreg_or_w
                sp.reg_mul(wreg, p, 2)
                if k:
                    sp.reg_add(wreg, wreg, 1)
                sp.reg_add(thr, wreg, -1)
                sp.reg_mul(thr, thr, w)
                sp.reg_alu(thr, thr, 0, op=ALU.max)
                sp.wait_ge(sem_d2, thr)
                if pool:
                    sp.wait_ge(sem_psq, thr)
                sp.reg_alu(widx, wreg, wpp - 1, op=ALU.bitwise_and)
                sp.reg_mul(soff, widx, w * 64)
                src0 = bass.AP(embd, soff, [[n_items * 64, 64], [1, w * 64]])
                src1 = bass.AP(embd, soff, [[n_items * 64, 64], [1, w * 64]])
                sp.reg_mul(thr, p, 64)
                sp.reg_add(thr, thr, 32 * k + 64)
                sp.dma_start(out=dst0, in_=src0).then_inc(dma_sem, 16)
                sp.wait_ge(dma_sem, thr)
                sp.reg_add(thr, thr, 16)
                sp.dma_start(out=dst1, in_=src1).then_inc(dma_sem, 16)
                sp.wait_ge(dma_sem, thr)

        # ================= SYNC (DMA) program =================
        # dma_sem increments are totally ordered (wait after each DMA) so a
        # cumulative wait implies every earlier DMA completed.
        sp.dma_start(out=sel0[:, :], in_=s0d[:, :]).then_inc(dma_sem, 16)
        sp.wait_ge(dma_sem, 16)
        sp.dma_start(out=sel1[:, :], in_=s1d[:, :]).then_inc(dma_sem, 16)
        sp.wait_ge(dma_sem, 32)
        sp.dma_start(out=G[:, :], in_=g0d[:, :]).then_inc(dma_sem, 16)
        sp.wait_ge(dma_sem, 48)

        if unroll:
            for wi in range(nw):
                emit_window_load(wi % 2, wi)
        else:
            with (
                sp.register("wreg") as wreg,
                sp.register("widx") as widx,
                sp.register("soff") as soff,
                sp.register("thr") as thr,
            ):
                with nc.Fori(0, npairs, engines=[SP]) as p:
                    for k in (0, 1):
                        emit_window_load(k, (wreg, widx, soff, thr, p))

        sp.wait_ge(sem_done, 1)
        sp.dma_start(out=outd[:, :], in_=osb[:, :]).then_inc(dma_sem, 16)
        dma_total = 48 + nw * 32 + 16
        sp.wait_ge(dma_sem, dma_total)
        if debug_steps is not None:
            dbg_srcs = {"dG": G, "ddiff": diff, "dsq": sq, "dd2": d2,
                        "dd2f": d2fsb, "dmt": mt, "dohp": ohp, "dt1": t1,
                        "dt2": t2, "dmkL": mkL}
            for nm, dram in dbg_outs.items():
                sp.dma_start(out=dram[:, :],
                             in_=dbg_srcs[nm][:, :]).then_inc(dma_sem, 16)
                dma_total += 16
                sp.wait_ge(dma_sem, dma_total)

        # ================= PE program =================
        def emit_pe_body():
            # fp32 selection matmuls: a 0/1 permutation through fp32r is
            # bitwise-lossless (hi+lo recombine exactly in PSUM).
            if one_mm:
                # sel0 holds selU: picks p = m%64 and p = 64+m%64; the
                # complement halves of d2w are 0, so out[m, j<8] = h0 value
                # and out[m, j>=8] = h1 value, exactly.
                pe.matmul(d2f[:, 0:16], sel0[0:128, :], d2[0:128, 0:16],
                          start=True, stop=True).then_inc(sem_sel, 1)
            else:
                pe.matmul(d2f[:, 0:8], sel0[0:64, :], d2[0:64, :],
                          start=True, stop=True)
                pe.matmul(d2f[:, 8:16], sel1[64:128, :], d2[64:128, :],
                          start=True, stop=True).then_inc(sem_sel, 1)
            # off-chain dummy matmuls keep the PE clock at a higher pstate
            for _ in range(pe_warm):
                pe.matmul(pwrm[:, :], sel1[64:128, :], d2[64:128, 0:8],
                          start=True, stop=True)

        if no_pe:
            pass
        elif unroll:
            for s in range(debug_steps if debug_steps is not None else nsteps):
                pe.wait_ge(sem_d2, s + 1)
                emit_pe_body()
        else:
            with pe.register("gsp") as gsp:
                pe.reg_mov(gsp, 0)
                with nc.Fori(0, nsteps, engines=[PE]):
                    pe.reg_add(gsp, gsp, 1)
                    pe.wait_ge(sem_d2, gsp)
                    emit_pe_body()

        # ================= DVE program =================
        v.wait_ge(dma_sem, 48)
        VC(v.memset(ohp[:, :], 0.0))
        VW(); VC(v.memset(s17[:, :], 0.0))
        VW(); VC(v.memset(eq17[:, :], 0.0))
        VW(); VC(v.memset(zz16[:, :], 0.0))
        if one_mm:
            VW(); VC(v.memset(d2[:, :], 0.0))
        if unroll:
            s = 0
            stop_at = debug_steps if debug_steps is not None else nsteps
            for wi in range(nw):
                if s >= stop_at:
                    break
                v.wait_ge(dma_sem, 48 + 32 * (wi + 1))
                base = (wi % 2) * (w * 64)
                for ti in range(w):
                    if s >= stop_at:
                        break
                    emit_step(base + ti * 64, s)
                    s += 1
            if debug_steps is not None:
                VW(); VC(v.tensor_copy(d2fsb[:, :], d2f[:, :]))
        else:
            with v.register("gsv") as gsv, v.register("thv") as thv:
                v.reg_mov(gsv, 0)
                with nc.Fori(0, npairs, engines=[DVE]) as p:
                    for k in (0, 1):
                        v.reg_mul(thv, p, 64)
                        v.reg_add(thv, thv, 32 * k + 80)
                        v.wait_ge(dma_sem, thv)
                        base = k * (w * 64)
                        with nc.Fori(base, base + w * 64, 64,
                                     engines=[DVE]) as eo:
                            emit_step(eo, gsv)
        VW()
        if pool:
            v.wait_ge(sem_pg, nsteps)
        v.tensor_reduce(osb[:, :], g_dn, axis=AX.X,
                        op=ALU.add).then_inc(sem_done, 1)

        # ================= POOL program (node-slice 6-7) =================
        if pool:
            gp = nc.gpsimd
            gp.wait_ge(dma_sem, 48)
            with gp.register("gpv") as gpv, gp.register("thp") as thp:
                gp.reg_mov(gpv, 0)
                with nc.Fori(0, npairs, engines=[POOLE]) as p:
                    for k in (0, 1):
                        gp.reg_mul(thp, p, 64)
                        gp.reg_add(thp, thp, 32 * k + 80)
                        gp.wait_ge(dma_sem, thp)
                        base = k * (w * 64)
                        with nc.Fori(base, base + w * 64, 64,
                                     engines=[POOLE]) as eo:
                            gp.reg_add(gpv, gpv, 1)
                            e_bP = bass.AP(E, eo, [[EW, 128], [0, npo],
                                                   [1, 64]])
                            gp.tensor_tensor(diff_3dP, e_bP, g_3dP,
                                             op=ALU.subtract)
                            gp.tensor_tensor(sq[:, NB:512], diff[:, NB:512],
                                             diff[:, NB:512],
                                             op=ALU.mult).then_inc(sem_psq, 1)
                            gp.wait_ge(sem_mk, gpv)
                            gp.tensor_tensor(upd_3dP, diff_3dP, mkL_bP,
                                             op=ALU.mult)
                            gp.tensor_tensor(G[:, NB:512], G[:, NB:512],
                                             sq[:, NB:512],
                                             op=ALU.add).then_inc(sem_pg, 1)

    return nc


def make_host_inputs(embeddings, nodes, n_items=T, one_mm=False):
    """Build per-core in_maps from the full inputs."""
    emb = np.ascontiguousarray(np.asarray(embeddings, dtype=np.float32))
    nodes = np.asarray(nodes, dtype=np.float32)
    nodes_flat = nodes.reshape(16, 64)

    g0 = np.empty((128, 512), dtype=np.float32)
    g0[0:64, :] = nodes_flat[0:8].reshape(512)[None, :]
    g0[64:128, :] = nodes_flat[8:16].reshape(512)[None, :]

    sel0 = np.zeros((128, 128), dtype=np.float32)
    sel1 = np.zeros((128, 128), dtype=np.float32)
    for m in range(128):
        if one_mm:
            sel0[m % 64, m] = 1.0
            sel0[64 + m % 64, m] = 1.0
        else:
            sel0[m % 64, m] = 1.0
        sel1[64 + m % 64, m] = 1.0

    ncores = emb.shape[0] // LANES
    in_maps = []
    for c in range(ncores):
        shard = emb[c * LANES:(c + 1) * LANES].reshape(LANES, n_items * 64)
        in_maps.append({
            "emb": np.ascontiguousarray(shard),
            "g0": g0,
            "sel0": sel0,
            "sel1": sel1,
        })
    return in_maps


def kernel(embeddings, nodes):
    from concourse.bass_utils import run_bass_kernel_spmd

    nc = build_nc()
    nc.detect_race_conditions = False
    in_maps = make_host_inputs(embeddings, nodes)
    res = run_bass_kernel_spmd(nc, in_maps, core_ids=list(range(NCORES)))
    out = np.empty((B, D), dtype=np.float32)
    for c in range(NCORES):
        o = np.asarray(res.results[c]["out"])
        out[c * LANES:(c + 1) * LANES] = o[0:64] + o[64:128]
    return out



# revision 27
# speedup vs baseline: 1.1300x; 1.1300x over previous
"""Trainium2 Bass kernel for the AGSOM sequential scan problem.

Problem: embeddings [512, 1024, 64], nodes [4,4,64]. For each batch lane, run
3 sequential passes over the 1024 items; each step finds the BMU (argmin
euclidean distance over the 16 grid nodes), then pulls the BMU's 4-connected
neighbors toward the item by LR=0.01. Output = per-lane sum of final grid
nodes -> [512, 64].

Sharding: pure data parallel, 64 batch lanes per NeuronCore across 8 cores.

Per-core layout ("layout B"): SBUF partition p = h*64 + b, where b is the
lane and h selects half of the grid (h=0: rows 0-1 = nodes 0..7, h=1: rows
2-3 = nodes 8..15). Grid state G is [128, 8*64] fp32. Each step:
  DVE:  diff = e - g; sq = diff^2; d2 = segmented reduce -> [128, 8]
  PE :  two selection matmuls gather both halves' d2 into PSUM [128, 16]
        (the only cross-partition exchange; lanes never mix)
  DVE:  m = min(d2full); oh = (d2full == m)*LR written into a zero-padded
        6x6 tile; 4-neighbor stencil via two shifted adds; per-half local
        mask; update g += mask * diff.
Items are DMA'd in double-buffered windows; the schedule runs under hardware
Fori loops with semaphore pipelining between DVE / PE / DMA. Same-engine
ordering relies on per-engine in-order execution (DVE drains between ops);
build with unroll=True, paranoid=True for a sim-checkable fully-synced graph.
"""
import numpy as np

LR = 0.01

# Fixed full-problem config
B, T, D = 512, 1024, 64
NCORES = 8
LANES = B // NCORES  # 64 lanes per core
NPASSES = 3
WIN = 128  # items per DMA window


def build_nc(n_items=T, n_passes=NPASSES, w=WIN, unroll=False, sync="mixd48",
             debug_steps=None, no_pe=False, pe_warm=0, pool=False,
             pool_nodes=2, one_mm=False, red_split=False, min_chain=False,
             xcopy=False, rsplit2=0):
    """sync: 'drain' (HW-correct engine-pipeline barriers), 'sem' (full
    semaphore chain; sim-race-detector-checkable; requires unroll), or
    'none' (no same-engine barriers; incorrect on HW, for perf probing)."""
    from contextlib import ExitStack
    from concourse import bass, mybir

    paranoid = sync == "sem"
    assert not (paranoid and not unroll), "sync='sem' requires unroll"
    assert debug_steps is None or unroll, "debug_steps requires unroll"

    f32 = mybir.dt.float32
    ALU = mybir.AluOpType
    AX = mybir.AxisListType
    DVE = mybir.EngineType.DVE
    PE = mybir.EngineType.PE
    SP = mybir.EngineType.SP
    POOLE = mybir.EngineType.Pool
    assert not (pool and unroll), "pool slicing implemented for Fori mode only"
    assert not (pool and no_pe)
    assert 1 <= pool_nodes <= 7
    assert not (red_split and not pool), "red_split only meaningful with pool"
    if xcopy:
        assert min_chain and not one_mm and not pool
        no_pe = True  # PE program unused; chain runs off the SBUF d2x tile
        # xcopy=2: node-ordered columns in both halves (same-column
        # partition-shifted copies, single one-hot op, no negative strides)

    wpp = n_items // w            # windows per pass
    assert wpp * w == n_items
    nw = n_passes * wpp           # total windows
    assert nw % 2 == 0, "need an even number of windows for the pair loop"
    npairs = nw // 2
    nsteps = n_passes * n_items
    EW = 2 * w * 64               # E row length (2 slots)

    nc = bass.Bass(target_bir_lowering=False)

    embd = nc.declare_dram_parameter("emb", [LANES, n_items * 64], f32, isOutput=False)
    g0d = nc.declare_dram_parameter("g0", [128, 512], f32, isOutput=False)
    s0d = nc.declare_dram_parameter("sel0", [128, 128], f32, isOutput=False)
    s1d = nc.declare_dram_parameter("sel1", [128, 128], f32, isOutput=False)
    outd = nc.declare_dram_parameter("out", [128, 64], f32, isOutput=True)
    dbg_outs = {}
    if debug_steps is not None:
        for nm, cols in [("dG", 512), ("ddiff", 512), ("dsq", 512),
                         ("dd2", 16 if (one_mm or xcopy) else 8),
                         ("dd2f", 16), ("dmt", 1), ("dohp", 36), ("dt1", 16),
                         ("dt2", 16), ("dmkL", 8)]:
            dbg_outs[nm] = nc.declare_dram_parameter(nm, [128, cols], f32,
                                                     isOutput=True)

    with ExitStack() as ctx:
        sem_d2 = ctx.enter_context(nc.semaphore("sem_d2"))
        sem_sel = ctx.enter_context(nc.semaphore("sem_sel"))
        sem_done = ctx.enter_context(nc.semaphore("sem_done"))
        dma_sem = ctx.enter_context(nc.semaphore("dma_sem"))
        sem_psq = ctx.enter_context(nc.semaphore("sem_psq")) if pool else None
        sem_mk = ctx.enter_context(nc.semaphore("sem_mk")) if pool else None
        sem_pg = ctx.enter_context(nc.semaphore("sem_pg")) if pool else None
        vch = ctx.enter_context(nc.semaphore("vch")) if paranoid else None
        esems = [ctx.enter_context(nc.semaphore(f"es{i}")) for i in range(6)] \
            if sync == "selfsem" else None

        E = ctx.enter_context(nc.sbuf_tensor("E", [128, EW], f32))
        G = ctx.enter_context(nc.sbuf_tensor("G", [128, 512], f32))
        diff = ctx.enter_context(nc.sbuf_tensor("diff", [128, 512], f32))
        sq = ctx.enter_context(nc.sbuf_tensor("sq", [128, 512], f32))
        d2 = ctx.enter_context(nc.sbuf_tensor(
            "d2", [128, 16 if (one_mm or xcopy) else 8], f32))
        d2p = ctx.enter_context(nc.sbuf_tensor("d2p", [128, 16], f32))
        mt = ctx.enter_context(nc.sbuf_tensor("mt", [128, 1], f32))
        ohp = ctx.enter_context(nc.sbuf_tensor("ohp", [128, 36], f32))
        t1 = ctx.enter_context(nc.sbuf_tensor("t1", [128, 16], f32))
        t2 = ctx.enter_context(nc.sbuf_tensor("t2", [128, 16], f32))
        mkL = ctx.enter_context(nc.sbuf_tensor("mkL", [128, 8], f32))
        s17 = ctx.enter_context(nc.sbuf_tensor("s17", [128, 17], f32))
        eq17 = ctx.enter_context(nc.sbuf_tensor("eq17", [128, 17], f32))
        zz16 = ctx.enter_context(nc.sbuf_tensor("zz16", [128, 16], f32))
        sel0 = ctx.enter_context(nc.sbuf_tensor("sel0sb", [128, 128], f32))
        sel1 = ctx.enter_context(nc.sbuf_tensor("sel1sb", [128, 128], f32))
        osb = ctx.enter_context(nc.sbuf_tensor("osb", [128, 64], f32))

        d2f = ctx.enter_context(nc.psum_tensor("d2f", [128, 16], f32))
        pwrm = ctx.enter_context(nc.psum_tensor("pwrm", [128, 8], f32))
        d2fsb = ctx.enter_context(nc.sbuf_tensor("d2fsb", [128, 16], f32))

        v = nc.vector
        pe = nc.tensor
        sp = nc.sync

        # ---- static APs ----
        g_3d = bass.AP(G, 0, [[512, 128], [64, 8], [1, 64]])
        g_2d = G[:, :]
        diff_3d = bass.AP(diff, 0, [[512, 128], [64, 8], [1, 64]])
        diff_2d = diff[:, :]
        sq_2d = sq[:, :]
        sq_3d = bass.AP(sq, 0, [[512, 128], [64, 8], [1, 64]])
        sq_4d = bass.AP(sq, 0, [[512, 128], [64, 8], [32, 2], [1, 32]])
        d2p_out = bass.AP(d2p, 0, [[16, 128], [2, 8], [1, 2]])
        d2p_a = bass.AP(d2p, 0, [[16, 128], [2, 8]])
        d2p_b = bass.AP(d2p, 1, [[16, 128], [2, 8]])
        upd_3d = sq_3d  # upd reuses the sq tile
        upd_2d = sq_2d
        mkL_b = bass.AP(mkL, 0, [[8, 128], [1, 8], [0, 64]])
        ohp_int = bass.AP(ohp, 7, [[36, 128], [6, 4], [1, 4]])
        d2f_44 = bass.AP(d2f, 0, [[16, 128], [4, 4], [1, 4]])
        d2f_2d = d2f[:, :]
        # stencil source views on the padded 6x6 tile
        s_y0 = bass.AP(ohp, 6, [[36, 128], [6, 4], [1, 4]])    # oh[x, y-1]
        s_y2 = bass.AP(ohp, 8, [[36, 128], [6, 4], [1, 4]])    # oh[x, y+1]
        s_x0 = bass.AP(ohp, 1, [[36, 128], [6, 4], [1, 4]])    # oh[x-1, y]
        s_x2 = bass.AP(ohp, 13, [[36, 128], [6, 4], [1, 4]])   # oh[x+1, y]
        t1_44 = bass.AP(t1, 0, [[16, 128], [4, 4], [1, 4]])
        t2_44 = bass.AP(t2, 0, [[16, 128], [4, 4], [1, 4]])
        t1_h0 = t1[0:64, 0:8]
        t2_h0 = t2[0:64, 0:8]
        mk_h0 = mkL[0:64, 0:8]
        t1_h1 = t1[64:128, 8:16]
        t2_h1 = t2[64:128, 8:16]
        mk_h1 = mkL[64:128, 0:8]
        g_dn = bass.AP(G, 0, [[512, 128], [1, 64], [64, 8]])
        # --- pool slicing: DVE owns nodes 0..nd-1, POOL owns nodes nd..7 ---
        nd = 8 - pool_nodes
        npo = pool_nodes
        NB = nd * 64
        g_3dD = bass.AP(G, 0, [[512, 128], [64, nd], [1, 64]])
        diff_3dD = bass.AP(diff, 0, [[512, 128], [64, nd], [1, 64]])
        upd_3dD = bass.AP(sq, 0, [[512, 128], [64, nd], [1, 64]])
        mkL_bD = bass.AP(mkL, 0, [[8, 128], [1, nd], [0, 64]])
        g_3dP = bass.AP(G, NB, [[512, 128], [64, npo], [1, 64]])
        diff_3dP = bass.AP(diff, NB, [[512, 128], [64, npo], [1, 64]])
        upd_3dP = bass.AP(sq, NB, [[512, 128], [64, npo], [1, 64]])
        mkL_bP = bass.AP(mkL, nd, [[8, 128], [1, npo], [0, 64]])
        # split-reduce APs: DVE-node segment and POOL-node segment
        d2p_outD = bass.AP(d2p, 0, [[16, 128], [2, nd], [1, 2]])
        sq_4dD = bass.AP(sq, 0, [[512, 128], [64, nd], [32, 2], [1, 32]])
        d2p_outP = bass.AP(d2p, 2 * nd, [[16, 128], [2, npo], [1, 2]])
        sq_4dP = bass.AP(sq, NB, [[512, 128], [64, npo], [32, 2], [1, 32]])
        # one_mm: split pair-adds into the zero-padded d2w tile (reuses d2)
        d2p_aH0 = bass.AP(d2p, 0, [[16, 64], [2, 8]])
        d2p_bH0 = bass.AP(d2p, 1, [[16, 64], [2, 8]])
        d2p_aH1 = bass.AP(d2p, 64 * 16, [[16, 64], [2, 8]])
        d2p_bH1 = bass.AP(d2p, 64 * 16 + 1, [[16, 64], [2, 8]])
        # rsplit2: two 4-node reduces + per-half per-node-group pair-adds
        # that chase the reduce write-back tails (within-segment summation
        # order unchanged -> bitwise-identical d2)
        d2p_outA = bass.AP(d2p, 0, [[16, 128], [2, 4], [1, 2]])
        sq_4dA = bass.AP(sq, 0, [[512, 128], [64, 4], [32, 2], [1, 32]])
        d2p_outB = bass.AP(d2p, 8, [[16, 128], [2, 4], [1, 2]])
        sq_4dB = bass.AP(sq, 256, [[512, 128], [64, 4], [32, 2], [1, 32]])
        d2p_aH0a = bass.AP(d2p, 0, [[16, 64], [2, 4]])
        d2p_bH0a = bass.AP(d2p, 1, [[16, 64], [2, 4]])
        d2p_aH1a = bass.AP(d2p, 64 * 16, [[16, 64], [2, 4]])
        d2p_bH1a = bass.AP(d2p, 64 * 16 + 1, [[16, 64], [2, 4]])
        d2p_aH0b = bass.AP(d2p, 8, [[16, 64], [2, 4]])
        d2p_bH0b = bass.AP(d2p, 9, [[16, 64], [2, 4]])
        d2p_aH1b = bass.AP(d2p, 64 * 16 + 8, [[16, 64], [2, 4]])
        d2p_bH1b = bass.AP(d2p, 64 * 16 + 9, [[16, 64], [2, 4]])
        # xcopy: partition-base-shifted views for the half exchange, and
        # per-half one-hot APs (h1 sees nodes in rotated order 8..15,0..7)
        d2x_h0own = bass.AP(d2, 0, [[16, 64], [1, 8]])
        d2x_h1own = bass.AP(d2, 64 * 16, [[16, 64], [1, 8]])
        d2x_h0oth = bass.AP(d2, 8, [[16, 64], [1, 8]])
        d2x_h1oth = bass.AP(d2, 64 * 16 + 8, [[16, 64], [1, 8]])
        d2x_2d = d2[:, :]
        d2x_h0_44 = bass.AP(d2, 0, [[16, 64], [4, 4], [1, 4]])
        d2x_h1_44 = bass.AP(d2, 64 * 16, [[16, 64], [8, 2], [4, 2], [1, 4]])
        d2x_44 = bass.AP(d2, 0, [[16, 128], [4, 4], [1, 4]])
        ohp_int_h0 = bass.AP(ohp, 7, [[36, 64], [6, 4], [1, 4]])
        ohp_int_h1 = bass.AP(ohp, 64 * 36 + 19,
                             [[36, 64], [-12, 2], [6, 2], [1, 4]])
        s17_out = s17[:, 1:17]
        eq17_a = bass.AP(eq17, 1, [[17, 128], [4, 4], [1, 4]])
        eq17_b = bass.AP(eq17, 0, [[17, 128], [4, 4], [1, 4]])
        INF0 = 3.0e38

        # Same-engine ordering between dependent DVE ops. On TRN2 hardware,
        # back-to-back DVE instructions can overlap in the pipe: a reader can
        # overtake the previous writer, so every RAW/WAR edge needs a barrier.
        # 'sem': then_inc/wait_ge chain (visible to the sim race detector).
        # 'drain': engine pipeline flush instruction (cheap, HW-correct).
        vcount = [0]

        def VC(instr, inc=True):
            if paranoid and inc:
                instr.then_inc(vch, 1)
                vcount[0] += 1
            return instr

        def VW(hazard=True):
            """hazard=True: real short-distance RAW edge (drain on HW).
            hazard=False: safe-by-streaming-lag on HW, but still a
            dependency edge the sim's race detector must see."""
            if paranoid and vcount[0] > 0:
                v.wait_ge(vch, vcount[0])
            elif sync == "drain" and hazard:
                v.drain()
            elif sync == "nop" and hazard:
                v.nop(cycle_cnt=160, nofuse=True)


        def emit_step(e_off, step_idx):
            """One SOM step. Barriers only on hazardous same-engine edges
            (short-distance RAW): either a drain, or in selfsem mode a
            then_inc on the producer + wait_ge(edge_sem, step+1) before the
            consumer. Long streaming ops reading the previous op's output in
            the same element order are safe with no barrier at all."""
            if not (unroll or no_pe):
                gsv = step_idx
                v.reg_add(gsv, gsv, 1)

            def HZ(instr, k):
                # producer side of hazard edge k
                if sync == "selfsem":
                    instr.then_inc(esems[k], 1)
                return instr

            def HW_(k):
                # consumer side of hazard edge k
                if paranoid and vcount[0] > 0:
                    v.wait_ge(vch, vcount[0])
                elif sync == "selfsem":
                    if unroll:
                        v.wait_ge(esems[k], step_idx + 1)
                    else:
                        v.wait_ge(esems[k], gsv)
                elif sync == "drain":
                    v.drain()
                elif sync.startswith("nopd"):
                    v.nop(cycle_cnt=int(sync[4:]), nofuse=True)
                elif sync.startswith("mixd"):
                    # full flush only after the 512-wide reduce (edge 0);
                    # short nop covers the small-op RAW edges
                    if k == 0:
                        v.drain()
                    else:
                        v.nop(cycle_cnt=int(sync[4:]), nofuse=True)
                elif sync.startswith("mix2_"):
                    # mix2_<A>_<B>: nop(A) after the reduce, nop(B) elsewhere
                    a, b = sync[5:].split("_")
                    v.nop(cycle_cnt=int(a) if k == 0 else int(b), nofuse=True)

            e_b = bass.AP(E, e_off, [[EW, 128], [0, 8], [1, 64]])
            if pool:
                e_bD = bass.AP(E, e_off, [[EW, 128], [0, nd], [1, 64]])
                VW(False); VC(v.tensor_tensor(diff_3dD, e_bD, g_3dD,
                                              op=ALU.subtract))
                VW(False); VC(v.tensor_tensor(sq[:, 0:NB], diff[:, 0:NB],
                                              diff[:, 0:NB], op=ALU.mult))
                if red_split:
                    # DVE-node reduce needs only DVE's sq; overlaps POOL tail
                    VW(False)
                    VC(v.tensor_reduce(d2p_outD, sq_4dD, axis=AX.X, op=ALU.add))
                    v.wait_ge(sem_psq, gsv)
                    VW(False)
                    HZ(VC(v.tensor_reduce(d2p_outP, sq_4dP, axis=AX.X,
                                          op=ALU.add)), 0)
                    HW_(0)
                else:
                    # full reduce reads POOL's sq slice too
                    v.wait_ge(sem_psq, gsv)
                    VW(False)
                    HZ(VC(v.tensor_reduce(d2p_out, sq_4d, axis=AX.X,
                                          op=ALU.add)), 0)
                    HW_(0)
            else:
                VW(False); VC(v.tensor_tensor(diff_3d, e_b, g_3d, op=ALU.subtract))
                VW(False); VC(v.tensor_tensor(sq_2d, diff_2d, diff_2d, op=ALU.mult))
                VW(False)
                HZ(VC(v.tensor_reduce(d2p_out, sq_4d, axis=AX.X, op=ALU.add)), 0)
                HW_(0)
            # sem_d2 inc carries this step's progress to PE and the DMA engine
            if xcopy == 2:
                # per-half pair sums into node-ordered columns, then two
                # same-column partition-base-shifted copies exchange halves
                VC(v.tensor_tensor(d2[0:64, 0:8], d2p_aH0, d2p_bH0,
                                   op=ALU.add))
                VW(False)
                v.tensor_tensor(d2[64:128, 8:16], d2p_aH1, d2p_bH1,
                                op=ALU.add).then_inc(sem_d2, 1)
                HW_(1)
                VC(v.tensor_copy(bass.AP(d2, 8, [[16, 64], [1, 8]]),
                                 bass.AP(d2, 64 * 16 + 8, [[16, 64], [1, 8]])))
                VW(False)
                VC(v.tensor_copy(bass.AP(d2, 64 * 16, [[16, 64], [1, 8]]),
                                 bass.AP(d2, 0, [[16, 64], [1, 8]])))
            elif xcopy:
                # own-half pair sums into cols 0:8, then partition-base-
                # shifted copies exchange the halves into cols 8:16 (h1's
                # view of the other half lands in rotated node order,
                # handled by the per-half one-hot APs below)
                v.tensor_tensor(d2[:, 0:8], d2p_a, d2p_b,
                                op=ALU.add).then_inc(sem_d2, 1)
                HW_(1)
                VC(v.tensor_copy(d2x_h0oth, d2x_h1own))
                VW(False)
                VC(v.tensor_copy(d2x_h1oth, d2x_h0own))
            elif one_mm:
                # split pair-adds land each half's 8 sums in its own column
                # range of the zero-padded d2w tile; one K=128 matmul then
                # gathers both halves (complement columns stay 0, so the
                # PSUM sum adds an exact +0.0)
                VC(v.tensor_tensor(d2[0:64, 0:8], d2p_aH0, d2p_bH0,
                                   op=ALU.add))
                VW(False)
                v.tensor_tensor(d2[64:128, 8:16], d2p_aH1, d2p_bH1,
                                op=ALU.add).then_inc(sem_d2, 1)
            else:
                v.tensor_tensor(d2[:, :], d2p_a, d2p_b,
                                op=ALU.add).then_inc(sem_d2, 1)
            if no_pe:
                pass
            elif unroll:
                v.wait_ge(sem_sel, step_idx + 1)
            else:
                v.wait_ge(sem_sel, gsv)
            if min_chain and xcopy == 2:
                VW(False)
                HW_(2)
                HZ(VC(v.tensor_reduce(mt[:, 0:1], d2x_2d, axis=AX.X,
                                      op=ALU.min)), 1)
                HW_(1)
                HZ(VC(v.tensor_scalar(ohp_int, d2x_44, mt[:, 0:1], LR,
                                      op0=ALU.is_equal, op1=ALU.mult)), 3)
                HW_(3)
            elif min_chain and xcopy:
                VW(False)
                HW_(2)
                HZ(VC(v.tensor_reduce(mt[:, 0:1], d2x_2d, axis=AX.X,
                                      op=ALU.min)), 1)
                HW_(1)
                VC(v.tensor_scalar(ohp_int_h0, d2x_h0_44, mt[0:64, 0:1], LR,
                                   op0=ALU.is_equal, op1=ALU.mult))
                VW(False)
                HZ(VC(v.tensor_scalar(ohp_int_h1, d2x_h1_44, mt[64:128, 0:1],
                                      LR, op0=ALU.is_equal, op1=ALU.mult)), 3)
                HW_(3)
            elif min_chain:
                # min-reduce + is_equal one-hot straight into the padded
                # tile. Ties (bitwise-equal d2, ~1 in 250k steps) multi-hot
                # instead of first-index - an extra argmin-flip-scale
                # perturbation, below the schedule-noise floor.
                VW(False)
                HZ(VC(v.tensor_reduce(mt[:, 0:1], d2f_2d, axis=AX.X,
                                      op=ALU.min)), 1)
                HW_(1)
                HZ(VC(v.tensor_scalar(ohp_int, d2f_44, mt[:, 0:1], LR,
                                      op0=ALU.is_equal, op1=ALU.mult)), 3)
                HW_(3)
            else:
                # running-min scan over the 16 gathered distances; s17[:,16]
                # is the global min, and the first position where the running
                # min equals it is the argmin with first-index tie-breaking.
                VW(False)
                HZ(VC(v.tensor_tensor_scan(s17_out, d2f_2d, zz16[:, :], INF0,
                                           op0=ALU.min, op1=ALU.bypass)), 1)
                HW_(1)
                HZ(VC(v.tensor_scalar(eq17[:, :], s17[:, :], s17[:, 16:17],
                                      LR, op0=ALU.is_equal, op1=ALU.mult)), 2)
                HW_(2)
                HZ(VC(v.tensor_tensor(ohp_int, eq17_a, eq17_b,
                                      op=ALU.subtract)), 3)
                HW_(3)
            VC(v.tensor_tensor(t1_44, s_y0, s_y2, op=ALU.add))
            VW(False)
            HZ(VC(v.tensor_tensor(t2_44, s_x0, s_x2, op=ALU.add)), 4)
            HW_(4)
            VC(v.tensor_tensor(mk_h0, t1_h0, t2_h0, op=ALU.add))
            VW(False)
            mk1i = HZ(VC(v.tensor_tensor(mk_h1, t1_h1, t2_h1, op=ALU.add)), 5)
            if pool:
                mk1i.then_inc(sem_mk, 1)
            HW_(5)
            if pool:
                VC(v.tensor_tensor(upd_3dD, diff_3dD, mkL_bD, op=ALU.mult))
                VW(False)
                VC(v.tensor_tensor(G[:, 0:NB], G[:, 0:NB], sq[:, 0:NB],
                                   op=ALU.add))
            else:
                VC(v.tensor_tensor(upd_3d, diff_3d, mkL_b, op=ALU.mult))
                # updadd reads upd in the same element order it was written,
                # with a full-op lag - streaming-safe without a drain
                VW(False)
                VC(v.tensor_tensor(g_2d, g_2d, upd_2d, op=ALU.add))

        def emit_window_load(k, wreg_or_w, pair=None):
            """SP-side load of window w into slot k (static k)."""
            dst0 = bass.AP(E, k * (w * 64), [[EW, 64], [1, w * 64]])
            dst1 = bass.AP(E, 64 * EW + k * (w * 64), [[EW, 64], [1, w * 64]])
            if unroll:
                wi = wreg_or_w
                sp.wait_ge(sem_d2, max(0, (wi - 1) * w))
                soff_i = (wi % wpp) * (w * 64)
                src0 = bass.AP(embd, soff_i, [[n_items * 64, 64], [1, w * 64]])
                src1 = bass.AP(embd, soff_i, [[n_items * 64, 64], [1, w * 64]])
                base = 48 + 32 * wi
                sp.dma_start(out=dst0, in_=src0).then_inc(dma_sem, 16)
                sp.wait_ge(dma_sem, base + 16)
                sp.dma_start(out=dst1, in_=src1).then_inc(dma_sem, 16)
                sp.wait_ge(dma_sem, base + 32)
            else:
                wreg, widx, soff, thr, p = wreg_or_w
                sp.reg_mul(wreg, p, 2)
                if k:
                    sp.reg_add(wreg, wreg, 1)
                sp.reg_add(thr, wreg, -1)
                sp.reg_mul(thr, thr, w)
                sp.reg_alu(thr, thr, 0, op=ALU.max)
                sp.wait_ge(sem_d2, thr)
                if pool:
                    sp.wait_ge(sem_psq, thr)
                sp.reg_alu(widx, wreg, wpp - 1, op=ALU.bitwise_and)
                sp.reg_mul(soff, widx, w * 64)
                src0 = bass.AP(embd, soff, [[n_items * 64, 64], [1, w * 64]])
                src1 = bass.AP(embd, soff, [[n_items * 64, 64], [1, w * 64]])
                sp.reg_mul(thr, p, 64)
                sp.reg_add(thr, thr, 32 * k + 64)
                sp.dma_start(out=dst0, in_=src0).then_inc(dma_sem, 16)
                sp.wait_ge(dma_sem, thr)
                sp.reg_add(thr, thr, 16)
                sp.dma_start(out=dst1, in_=src1).then_inc(dma_sem, 16)
                sp.wait_ge(dma_sem, thr)

        # ================= SYNC (DMA) program =================
        # dma_sem increments are totally ordered (wait after each DMA) so a
        # cumulative wait implies every earlier DMA completed.
        sp.dma_start(out=sel0[:, :], in_=s0d[:, :]).then_inc(dma_sem, 16)
        sp.wait_ge(dma_sem, 16)
        sp.dma_start(out=sel1[:, :], in_=s1d[:, :]).then_inc(dma_sem, 16)
        sp.wait_ge(dma_sem, 32)
        sp.dma_start(out=G[:, :], in_=g0d[:, :]).then_inc(dma_sem, 16)
        sp.wait_ge(dma_sem, 48)

        if unroll:
            for wi in range(nw):
                emit_window_load(wi % 2, wi)
        else:
            with (
                sp.register("wreg") as wreg,
                sp.register("widx") as widx,
                sp.register("soff") as soff,
                sp.register("thr") as thr,
            ):
                with nc.Fori(0, npairs, engines=[SP]) as p:
                    for k in (0, 1):
                        emit_window_load(k, (wreg, widx, soff, thr, p))

        sp.wait_ge(sem_done, 1)
        sp.dma_start(out=outd[:, :], in_=osb[:, :]).then_inc(dma_sem, 16)
        dma_total = 48 + nw * 32 + 16
        sp.wait_ge(dma_sem, dma_total)
        if debug_steps is not None:
            dbg_srcs = {"dG": G, "ddiff": diff, "dsq": sq, "dd2": d2,
                        "dd2f": d2fsb, "dmt": mt, "dohp": ohp, "dt1": t1,
                        "dt2": t2, "dmkL": mkL}
            for nm, dram in dbg_outs.items():
                sp.dma_start(out=dram[:, :],
                             in_=dbg_srcs[nm][:, :]).then_inc(dma_sem, 16)
                dma_total += 16
                sp.wait_ge(dma_sem, dma_total)

        # ================= PE program =================
        def emit_pe_body():
            # fp32 selection matmuls: a 0/1 permutation through fp32r is
            # bitwise-lossless (hi+lo recombine exactly in PSUM).
            if one_mm:
                # sel0 holds selU: picks p = m%64 and p = 64+m%64; the
                # complement halves of d2w are 0, so out[m, j<8] = h0 value
                # and out[m, j>=8] = h1 value, exactly.
                pe.matmul(d2f[:, 0:16], sel0[0:128, :], d2[0:128, 0:16],
                          start=True, stop=True).then_inc(sem_sel, 1)
            else:
                pe.matmul(d2f[:, 0:8], sel0[0:64, :], d2[0:64, :],
                          start=True, stop=True)
                pe.matmul(d2f[:, 8:16], sel1[64:128, :], d2[64:128, :],
                          start=True, stop=True).then_inc(sem_sel, 1)
            # off-chain dummy matmuls keep the PE clock at a higher pstate
            for _ in range(pe_warm):
                pe.matmul(pwrm[:, :], sel1[64:128, :], d2[64:128, 0:8],
                          start=True, stop=True)

        if no_pe:
            pass
        elif unroll:
            for s in range(debug_steps if debug_steps is not None else nsteps):
                pe.wait_ge(sem_d2, s + 1)
                emit_pe_body()
        else:
            with pe.register("gsp") as gsp:
                pe.reg_mov(gsp, 0)
                with nc.Fori(0, nsteps, engines=[PE]):
                    pe.reg_add(gsp, gsp, 1)
                    pe.wait_ge(sem_d2, gsp)
                    emit_pe_body()

        # ================= DVE program =================
        v.wait_ge(dma_sem, 48)
        VC(v.memset(ohp[:, :], 0.0))
        VW(); VC(v.memset(s17[:, :], 0.0))
        VW(); VC(v.memset(eq17[:, :], 0.0))
        VW(); VC(v.memset(zz16[:, :], 0.0))
        if one_mm:
            VW(); VC(v.memset(d2[:, :], 0.0))
        if unroll:
            s = 0
            stop_at = debug_steps if debug_steps is not None else nsteps
            for wi in range(nw):
                if s >= stop_at:
                    break
                v.wait_ge(dma_sem, 48 + 32 * (wi + 1))
                base = (wi % 2) * (w * 64)
                for ti in range(w):
                    if s >= stop_at:
                        break
                    emit_step(base + ti * 64, s)
                    s += 1
            if debug_steps is not None:
                VW(); VC(v.tensor_copy(d2fsb[:, :], d2f[:, :]))
        else:
            with v.register("gsv") as gsv, v.register("thv") as thv:
                v.reg_mov(gsv, 0)
                with nc.Fori(0, npairs, engines=[DVE]) as p:
                    for k in (0, 1):
                        v.reg_mul(thv, p, 64)
                        v.reg_add(thv, thv, 32 * k + 80)
                        v.wait_ge(dma_sem, thv)
                        base = k * (w * 64)
                        with nc.Fori(base, base + w * 64, 64,
                                     engines=[DVE]) as eo:
                            emit_step(eo, gsv)
        VW()
        if pool:
            v.wait_ge(sem_pg, nsteps)
        v.tensor_reduce(osb[:, :], g_dn, axis=AX.X,
                        op=ALU.add).then_inc(sem_done, 1)

        # ================= POOL program (node-slice 6-7) =================
        if pool:
            gp = nc.gpsimd
            gp.wait_ge(dma_sem, 48)
            with gp.register("gpv") as gpv, gp.register("thp") as thp:
                gp.reg_mov(gpv, 0)
                with nc.Fori(0, npairs, engines=[POOLE]) as p:
                    for k in (0, 1):
                        gp.reg_mul(thp, p, 64)
                        gp.reg_add(thp, thp, 32 * k + 80)
                        gp.wait_ge(dma_sem, thp)
                        base = k * (w * 64)
                        with nc.Fori(base, base + w * 64, 64,
                                     engines=[POOLE]) as eo:
                            gp.reg_add(gpv, gpv, 1)
                            e_bP = bass.AP(E, eo, [[EW, 128], [0, npo],
                                                   [1, 64]])
                            gp.tensor_tensor(diff_3dP, e_bP, g_3dP,
                                             op=ALU.subtract)
                            gp.tensor_tensor(sq[:, NB:512], diff[:, NB:512],
                                             diff[:, NB:512],
                                             op=ALU.mult).then_inc(sem_psq, 1)
                            gp.wait_ge(sem_mk, gpv)
                            gp.tensor_tensor(upd_3dP, diff_3dP, mkL_bP,
                                             op=ALU.mult)
                            gp.tensor_tensor(G[:, NB:512], G[:, NB:512],
                                             sq[:, NB:512],
                                             op=ALU.add).then_inc(sem_pg, 1)

    return nc


def make_host_inputs(embeddings, nodes, n_items=T, one_mm=False):
    """Build per-core in_maps from the full inputs."""
    emb = np.ascontiguousarray(np.asarray(embeddings, dtype=np.float32))
    nodes = np.asarray(nodes, dtype=np.float32)
    nodes_flat = nodes.reshape(16, 64)

    g0 = np.empty((128, 512), dtype=np.float32)
    g0[0:64, :] = nodes_flat[0:8].reshape(512)[None, :]
    g0[64:128, :] = nodes_flat[8:16].reshape(512)[None, :]

    sel0 = np.zeros((128, 128), dtype=np.float32)
    sel1 = np.zeros((128, 128), dtype=np.float32)
    for m in range(128):
        if one_mm:
            sel0[m % 64, m] = 1.0
            sel0[64 + m % 64, m] = 1.0
        else:
            sel0[m % 64, m] = 1.0
        sel1[64 + m % 64, m] = 1.0

    ncores = emb.shape[0] // LANES
    in_maps = []
    for c in range(ncores):
        shard = emb[c * LANES:(c + 1) * LANES].reshape(LANES, n_items * 64)
        in_maps.append({
            "emb": np.ascontiguousarray(shard),
            "g0": g0,
            "sel0": sel0,
            "sel1": sel1,
        })
    return in_maps


def kernel(embeddings, nodes):
    from concourse.bass_utils import run_bass_kernel_spmd

    nc = build_nc()
    nc.detect_race_conditions = False
    in_maps = make_host_inputs(embeddings, nodes)
    res = run_bass_kernel_spmd(nc, in_maps, core_ids=list(range(NCORES)))
    out = np.empty((B, D), dtype=np.float32)
    for c in range(NCORES):
        o = np.asarray(res.results[c]["out"])
        out[c * LANES:(c + 1) * LANES] = o[0:64] + o[64:128]
    return out

